# revision 26
# baseline (speedup 1.0000x reference)
"""BertSelfAttention on 8 Trainium2 NeuronCores.

Sharding: 8 cores = 4 batches x 2 head-halves. Each core computes, for its
batch b and its 8 heads, the unnormalized attention output transposed
(out.T = V.T @ P.T per head) plus the softmax denominator row (via a ones
column appended to V). The host pre-transposes inputs (X.T, W.T slices,
cast to fp16) and does the final normalize/transpose/concat.

Pipeline (one head-qb block at a time, 16 blocks of 16 key-steps):
- per step, the two q-half score matmuls of the block's head stream
  concurrently on disjoint PE row groups (parity-swapped K/Q copies);
- exp is split ScalarE (exact, ~60%) / VectorE (Schraudolph fp16
  bit-trick via fused mult-add into int16, ~40%);
- AV trails the scores by AV_LAG steps;
- Q/K/V projection matmuls stream through two dedicated PSUM slots at a
  per-step budget (8/step while V builds during block 0, ~1/step in
  steady state), so the PE never runs multi-microsecond projection
  bursts that would starve the exp engines.
PSUM: 4 rotating [128,512] score banks (2 per step, 2-step elasticity),
2 banks for the AV accumulator, 2 banks for the projection slots.
"""

import sys

if "/opt/trn_rl_repo" not in sys.path:
    sys.path.insert(0, "/opt/trn_rl_repo")

import numpy as np

import concourse.bass as bass  # noqa: F401  (registers bass machinery)
import concourse.tile as tile
from concourse import bacc, mybir
from concourse.bass_utils import run_bass_kernel_spmd

B, S, H = 4, 2048, 1024
NH, DH = 16, 64
NCORES = 8
HPC = 8            # heads per core
OC = HPC * DH      # 512 output features per core
HC = H // 128      # 8 contraction chunks of 128
DHE = DH + 1       # head dim + denominator column

F16 = mybir.dt.float16
F32 = mybir.dt.float32
I16 = mybir.dt.int16
EXP = mybir.ActivationFunctionType.Exp

# DVE fast-exp (Schraudolph bit-trick, fp16 target): for score s,
# exp(s/8) ~= bits_as_fp16(round(SCH_A*s + SCH_B)). DVE converts
# fp32->int16 with round-to-nearest (HW-probed); C=60 zeroes the mean
# relative error (rms ~1.8% per element, ~1.1% on the final output at
# a ~40% tile share). Offloads exp work from the saturated ScalarE.
SCH_A = float(1024.0 / np.log(2.0) * 0.125)
SCH_B = 15360.0 - 60.0
AV_LAG = 3         # k-steps the AV matmuls trail the score matmuls

_PROGRAM = None
LAST_RESULT = None  # BassKernelResults of the most recent kernel() call


def _emit_kernel(tc, out, xt, wqt, wkt, wvt):
    nc = tc.nc
    with (
        tc.tile_pool(name="persist", bufs=1) as persist,
        tc.tile_pool(name="ptp", bufs=8) as ptp,
        tc.tile_pool(name="ost", bufs=4) as ost,
        tc.tile_pool(name="psa", bufs=1, space="PSUM") as psa,
    ):
        xt_sb = persist.tile([128, HC, S], F16)
        wq_sb = persist.tile([128, HC, OC], F16)
        wk_sb = persist.tile([128, HC, OC], F16)
        wv_sb = persist.tile([128, HC, OC], F16)
        qt_sb = persist.tile([128, 4, S], F16)
        kt_sb = persist.tile([128, 4, S], F16)
        # parity-swapped duplicates: head rows 0-63 in qt_sb sit at rows
        # 64-127 here (and vice versa), so a head's two q-half score matmuls
        # target disjoint PE row groups and stream concurrently.
        qt2_sb = persist.tile([128, 4, S], F16)
        kt2_sb = persist.tile([128, 4, S], F16)
        v_sb = persist.tile([128, 16, HPC * DHE], F16)

        # The ~7MB input load gates the first projections, so spread it over
        # all three DMA paths (SP + ScalarE hardware DGE queues, GpSimd
        # software DGE) instead of serializing ~20us on one queue. wv rides
        # gpsimd alone: it is not needed until block 0's V tiles.
        xt_chunks = xt.rearrange("(c p) s -> p c s", p=128)
        wv_chunks = wvt.rearrange("(c p) o -> p c o", p=128)
        # Few, large DMAs: the runtime only keeps ~4 DMA instructions in
        # flight per queue before chaining waits, so many small DMAs would
        # trickle in tens of microseconds late.
        nc.scalar.dma_start(wk_sb[:], wkt.rearrange("(c p) o -> p c o", p=128))
        nc.sync.dma_start(wq_sb[:], wqt.rearrange("(c p) o -> p c o", p=128))
        nc.sync.dma_start(xt_sb[:, 0:2, :], xt_chunks[:, 0:2, :])
        nc.scalar.dma_start(xt_sb[:, 2:4, :], xt_chunks[:, 2:4, :])
        nc.sync.dma_start(xt_sb[:, 4:6, :], xt_chunks[:, 4:6, :])
        nc.scalar.dma_start(xt_sb[:, 6:8, :], xt_chunks[:, 6:8, :])
        nc.gpsimd.dma_start(wv_sb[:, 0:4, :], wv_chunks[:, 0:4, :])
        nc.gpsimd.dma_start(wv_sb[:, 4:8, :], wv_chunks[:, 4:8, :])

        # HAM pre-warm: the PE clock-gate defaults to 1.2 GHz and only opens
        # to 2.4 GHz after ~3.4us of sustained matmul activity. Run throwaway
        # matmuls (on a small quickly-memset tile) while the input DMAs
        # stream, so the first real projections run at full clock.
        warm_sb = persist.tile([128, 512], F16)
        nc.vector.memset(warm_sb[:], 0.0)
        warm = psa.tile([128, 512], F32, tag="pp0", name="warm")
        for i in range(64):
            nc.tensor.matmul(
                warm[:], warm_sb[:, 0:128], warm_sb[:, 0:512],
                start=True, stop=True, skip_group_check=True,
            )

        # fill V with ones first; projection copies overwrite the data columns,
        # leaving a ones column per head to accumulate softmax denominators
        nc.vector.memset(v_sb[:], 1.0)

        # ---- projection job system ----
        # A job is one [128,512] projection tile: 8 accumulating matmuls +
        # one PSUM->SBUF cast (+ the parity-swap DMAs). Jobs stream through
        # two dedicated PSUM slots (pp0/pp1) at a per-step matmul budget so
        # projection work interleaves finely with attention matmuls.
        class Job:
            __slots__ = ("pp", "mm", "fin")

            def __init__(self, w_or_x, kind, c_or_st, sc):
                slot = Job.next_slot
                Job.next_slot ^= 1
                self.pp = psa.tile(
                    [128, 512], F32, tag=f"pp{slot}", name=f"pp{slot}"
                )
                if kind == "v":
                    st = c_or_st
                    self.mm = [
                        (
                            xt_sb[:, hc, st * 128 : (st + 1) * 128],
                            wv_sb[:, hc, :],
                            hc,
                        )
                        for hc in range(HC)
                    ]

                    def fin(st=st):
                        nc.vector.tensor_copy(
                            v_sb[:, st, :].rearrange("p (h e) -> p h e", e=DHE)[
                                :, :, 0:DH
                            ],
                            self.pp[:].rearrange("p (h d) -> p h d", d=DH),
                        )

                    self.fin = fin
                else:
                    c = c_or_st
                    w_sb = w_or_x
                    dst = kt_sb if kind == "k" else qt_sb
                    self.mm = [
                        (
                            w_sb[:, hc, c * 128 : (c + 1) * 128],
                            xt_sb[:, hc, sc * 512 : (sc + 1) * 512],
                            hc,
                        )
                        for hc in range(HC)
                    ]

                    def fin(c=c, sc=sc, kind=kind, dst=dst):
                        nc.vector.tensor_copy(
                            dst[:, c, sc * 512 : (sc + 1) * 512], self.pp[:]
                        )
                        lo, hi = sc * 512, (sc + 1) * 512
                        if kind == "k":
                            nc.sync.dma_start(
                                kt2_sb[0:64, c, lo:hi], kt_sb[64:128, c, lo:hi]
                            )
                            nc.sync.dma_start(
                                kt2_sb[64:128, c, lo:hi], kt_sb[0:64, c, lo:hi]
                            )
                        elif sc % 2 == 1:  # only odd q-halves feed the q2=1 path
                            nc.sync.dma_start(
                                qt2_sb[0:64, c, lo:hi], qt_sb[64:128, c, lo:hi]
                            )
                            nc.sync.dma_start(
                                qt2_sb[64:128, c, lo:hi], qt_sb[0:64, c, lo:hi]
                            )

                    self.fin = fin

            def emit_mms(self, n):
                while n > 0 and self.mm:
                    lhsT, rhs, hc = self.mm.pop(0)
                    nc.tensor.matmul(
                        self.pp[:], lhsT, rhs,
                        start=(hc == 0), stop=(hc == HC - 1),
                    )
                    n -= 1
                if not self.mm and self.fin is not None:
                    self.fin()
                    self.fin = None
                return n

            def done(self):
                return not self.mm and self.fin is None

        Job.next_slot = 0

        class ProjStream:
            def __init__(self):
                self.queue = []
                self.active = []

            def push(self, *jobs):
                self.queue.extend(jobs)

            def run(self, budget):
                while budget > 0:
                    while len(self.active) < 2 and self.queue:
                        self.active.append(self.queue.pop(0)())
                    if not self.active:
                        return
                    per = (budget + len(self.active) - 1) // len(self.active)
                    for j in list(self.active):
                        take = min(per, budget)
                        budget -= take - j.emit_mms(take)
                        if j.done():
                            self.active.remove(j)
                    if not self.queue and not self.active:
                        return

            def drain(self):
                self.run(10**9)

        stream = ProjStream()

        def kj(c, sc):
            return lambda: Job(wk_sb, "k", c, sc)

        def qj(c, sc):
            return lambda: Job(wq_sb, "q", c, sc)

        def vj(st):
            return lambda: Job(None, "v", st, 0)

        # ---- exp engine assignment: 50/50 ScalarE (exact) / DVE ----
        # one tile per engine per k-step, alternating q-half, so both exp
        # engines stay evenly fed (output copies ride ScalarE instead)
        def use_dve(k, q2):
            return (k + q2) % 2 == 0

        # ---- one attention block: (head, q-block) x 16 key tiles ----
        ps_rr = [0]

        def attention_block(head, qb, budget):
            chunk, par = head // 2, head % 2
            hsl = slice(head * DHE, (head + 1) * DHE)
            po = psa.tile([DHE, 1024], F32, tag="po", name="po")

            def av(k, pts):
                for q2 in range(2):
                    rhs = pts[q2][:]
                    if rhs.dtype == I16:
                        rhs = rhs.bitcast(F16)
                    nc.tensor.matmul(
                        po[:, q2 * 512 : (q2 + 1) * 512],
                        v_sb[:, k, hsl],
                        rhs,
                        start=(k == 0),
                        stop=(k == 15),
                    )

            pending = []
            for k in range(16):
                ksl = slice(k * 128, (k + 1) * 128)
                pts = {}
                for q2 in range(2):
                    q0 = qb * 1024 + q2 * 512
                    kt_src = kt_sb if q2 == 0 else kt2_sb
                    qt_src = qt_sb if q2 == 0 else qt2_sb
                    base = (par if q2 == 0 else 1 - par) * 64
                    t = psa.tile(
                        [128, 512], F32,
                        tag=f"ps{ps_rr[0] % 4}", name=f"s{ps_rr[0] % 4}",
                    )
                    ps_rr[0] += 1
                    nc.tensor.matmul(
                        t[:],
                        kt_src[base : base + 64, chunk, ksl],
                        qt_src[base : base + 64, chunk, q0 : q0 + 512],
                        start=True,
                        stop=True,
                    )
                    pts[q2] = t
                for q2 in range(2):
                    dve = use_dve(k, q2)
                    pt = ptp.tile(
                        [128, 512], I16 if dve else F16,
                        tag=f"pt{q2}", name=f"p{q2}",
                    )
                    if dve:
                        nc.vector.tensor_scalar(
                            pt[:], pts[q2][:], SCH_A, SCH_B,
                            mybir.AluOpType.mult, mybir.AluOpType.add,
                        )
                    else:
                        nc.scalar.activation(pt[:], pts[q2][:], EXP, scale=0.125)
                    pts[q2] = pt
                pending.append((k, pts))
                if len(pending) > AV_LAG:
                    av(*pending.pop(0))
                stream.run(budget[k])
            for item in pending:
                av(*item)
            o = ost.tile([DHE, 1024], F32, tag="o")
            for h2 in range(2):
                hs = slice(h2 * 512, (h2 + 1) * 512)
                nc.scalar.copy(o[:, hs], po[:, hs])
                nc.sync.dma_start(
                    out[head, :, qb * 1024 + h2 * 512 : qb * 1024 + (h2 + 1) * 512],
                    o[:, hs],
                )

        # ---- schedule ----
        # upfront (overlapped with the input DMAs): just enough of chunk 0
        # for block (head 0, qb 0) to start: K(0,0), Q(0,0), Q(0,1).
        for mk in (kj(0, 0), qj(0, 0), qj(0, 1)):
            j = mk()
            j.emit_mms(HC)

        # block 0 carries all 16 V tiles plus the remaining chunk-0 K tiles,
        # interleaved so K(0,sc) lands just before the k=4*sc score matmuls
        # need it.
        stream.push(
            vj(0), kj(0, 1), vj(1), kj(0, 2), vj(2), kj(0, 3),
            *[vj(s) for s in range(3, 16)],
        )
        attention_block(0, 0, [12] * 6 + [10] * 10)
        # block 1 (head 1, qb 0): finish chunk-0 Q + start chunk 1
        stream.push(qj(0, 2), qj(0, 3), kj(1, 0), kj(1, 1))
        attention_block(1, 0, [4] * 8 + [1] * 8)
        stream.push(kj(1, 2), kj(1, 3), qj(1, 0), qj(1, 1))
        attention_block(0, 1, [1, 2] * 8)
        attention_block(1, 1, [1, 2] * 8)

        # steady state: during chunk c's four blocks, finish Q(c,2|3) (needed
        # by its own qb1 blocks) and build chunk c+1's K and first-half Q.
        for c in (1, 2, 3):
            stream.push(qj(c, 2), qj(c, 3))
            if c < 3:
                stream.push(
                    kj(c + 1, 0), kj(c + 1, 1), kj(c + 1, 2), kj(c + 1, 3),
                    qj(c + 1, 0), qj(c + 1, 1),
                )
            budget = [1, 2] * 8 if c < 3 else [1] * 8 + [0] * 8
            attention_block(2 * c, 0, budget)
            attention_block(2 * c + 1, 0, budget)
            attention_block(2 * c, 1, [1] * 16 if c < 3 else [0] * 16)
            attention_block(2 * c + 1, 1, [0] * 16)
        stream.drain()


def _get_program():
    global _PROGRAM
    if _PROGRAM is None:
        nc = bacc.Bacc(
            "TRN2", target_bir_lowering=False, debug=False, num_devices=NCORES
        )
        xt = nc.dram_tensor("xt", [H, S], F16, kind="ExternalInput").ap()
        wqt = nc.dram_tensor("wqt", [H, OC], F16, kind="ExternalInput").ap()
        wkt = nc.dram_tensor("wkt", [H, OC], F16, kind="ExternalInput").ap()
        wvt = nc.dram_tensor("wvt", [H, OC], F16, kind="ExternalInput").ap()
        out = nc.dram_tensor("out", [HPC, DHE, S], F32, kind="ExternalOutput").ap()
        with tile.TileContext(nc) as tc:
            _emit_kernel(tc, out, xt, wqt, wkt, wvt)
        nc.compile()
        _PROGRAM = nc
    return _PROGRAM


def kernel(**inputs):
    global LAST_RESULT
    X = np.asarray(inputs["hidden_states"], dtype=np.float32)
    Ws = {k: np.asarray(inputs[k], dtype=np.float32) for k in ("Wq", "Wk", "Wv")}

    nc = _get_program()
    in_maps = []
    for core in range(NCORES):
        b, half = core // 2, core % 2
        sl = slice(half * OC, (half + 1) * OC)
        in_maps.append(
            {
                "xt": np.ascontiguousarray(X[b].T).astype(np.float16),
                "wqt": np.ascontiguousarray(Ws["Wq"][sl].T).astype(np.float16),
                "wkt": np.ascontiguousarray(Ws["Wk"][sl].T).astype(np.float16),
                "wvt": np.ascontiguousarray(Ws["Wv"][sl].T).astype(np.float16),
            }
        )

    LAST_RESULT = run_bass_kernel_spmd(nc, in_maps, core_ids=list(range(NCORES)))

    out = np.empty((B, S, H), dtype=np.float32)
    for core in range(NCORES):
        r = LAST_RESULT.results[core]["out"]          # [HPC, DHE, S]
        num = r[:, :DH, :]                            # [8, 64, 2048]
        den = r[:, DH : DH + 1, :]                    # [8, 1, 2048]
        o = (num / den).transpose(2, 0, 1).reshape(S, OC)
        b, half = core // 2, core % 2
        out[b, :, half * OC : (half + 1) * OC] = o
    return out
